# revision 6
# baseline (speedup 1.0000x reference)
"""Trainium2 Bass kernel for nn_BasicBlock (binary activation + binarized
weight-standardized 3x3 conv + residual + PReLU).

Contract: kernel(**inputs) takes FULL unsharded numpy inputs (keys as in
setup_inputs) and returns the FULL [32, 512, 28, 28] float32 output.
Internally shards the batch dim across 8 NeuronCores (4 images each); the
small conv weight + per-channel vectors are replicated.

Key math facts exploited:
- forward activations are sign(x*beta+b0) in {-1,0,1} and forward weights
  are sf[o]*gain[o]*sign(w_std) with sign in {-1,0,1}, so the conv
  contraction is exact in fp8 (products are +-1, fp32 PSUM accumulation);
  the per-channel scalar alpha*sf*gain folds into the epilogue.
- fp8e4 DoubleRow packs two contraction rows per PE cell (2 cin chunks per
  matmul), halving the matmul count. The DoubleRow LDWEIGHTS (256 cols,
  ~213ns) gates the matmul stream, so the conv phase runs at ~213ns/matmul;
  everything else must hide underneath it:
  * weight transposes go through the DMA XBAR (bf16) instead of the PE,
  * the epilogue is 3 ops (DVE/GpSimd/DVE) using PReLU(v)=max(v, a*v),
  * ACT does only signs + the |w-mean| accumulation pass.
"""

import numpy as np

import concourse.bass as bass
import concourse.mybir as mybir
import concourse.tile as tile
from concourse import bacc

# problem constants (hardcoded per harness contract)
N_CORES = 8
N_PER = 4          # images per core (32 / 8)
C = 512            # Cin == Cout
H = W = 28
HP = WP = 30       # zero-padded spatial
TAPS = 9
KFAN = C * TAPS    # 4608 = fan-in per output channel
ALPHA = 0.2
BETA = 1.0
EPS = 1e-5
WS_SCALE = 1.0 / float(np.sqrt(KFAN))  # fan_in**-0.5
NCH = C // 128     # 4 channel chunks of 128
NPAIR = NCH // 2   # 2 DoubleRow pairs of chunks
ROWS_PER_TILE = 14 # output rows per matmul tile
NSPAT = H // ROWS_PER_TILE  # 2 spatial tiles per image
NFREE = ROWS_PER_TILE * WP  # 420: contiguous run incl. 2 pad cols per row
ACT_IMG = 912  # padded 30x30 image (900) + 12 slack: %16==0 for DoubleRow,
               # and covers the last tile's 420-run overhang (482+420=902)

FP32 = mybir.dt.float32
BF16 = mybir.dt.bfloat16
FP8 = mybir.dt.float8e4


def _load_chunked_vec(nc, pool, dram_ap, name):
    """Load a [512] per-channel vector as 4 SBUF tiles of [128, 1]."""
    tiles = []
    for c in range(NCH):
        t = pool.tile([128, 1], FP32, tag=f"{name}_{c}", name=f"{name}_{c}")
        sl = dram_ap[c * 128 : (c + 1) * 128].rearrange("(p o) -> p o", o=1)
        nc.gpsimd.dma_start(out=t, in_=sl)
        tiles.append(t)
    return tiles


def build_program():
    nc = bacc.Bacc(
        "TRN2",
        target_bir_lowering=False,
        debug=False,
        num_devices=1,
        num_swdge_queues=4,
    )
    x_h = nc.declare_dram_parameter("x", [N_PER, C, H, W], FP32, isOutput=False)
    w_h = nc.declare_dram_parameter("conv_weight", [C, C, 3, 3], FP32, isOutput=False)
    gain_h = nc.declare_dram_parameter("gain", [C], FP32, isOutput=False)
    b0_h = nc.declare_dram_parameter("move0_bias", [C], FP32, isOutput=False)
    b1_h = nc.declare_dram_parameter("move1_bias", [C], FP32, isOutput=False)
    pa_h = nc.declare_dram_parameter("prelu_a", [C], FP32, isOutput=False)
    b2_h = nc.declare_dram_parameter("move2_bias", [C], FP32, isOutput=False)
    out_h = nc.declare_dram_parameter("out", [N_PER, C, H, W], FP32, isOutput=True)

    x_ap = x_h[:, :, :, :]
    w_ap = w_h[:, :, :, :]
    out_ap = out_h[:, :, :, :]

    with tile.TileContext(nc) as tc:
        with (
            tc.tile_pool(name="persist", bufs=1) as persist,
            tc.tile_pool(name="wraw", bufs=2) as wraw,
            tc.tile_pool(name="wsig", bufs=2) as wsig,
            tc.tile_pool(name="wst", bufs=6) as wst,
            tc.tile_pool(name="stats", bufs=4) as stats,
            tc.tile_pool(name="epi", bufs=4) as epi,
            tc.tile_pool(name="opool", bufs=3) as opool,
            tc.tile_pool(name="psum_mm", bufs=8, space="PSUM") as psum_mm,
        ):
            # ---- small per-channel vectors -------------------------------
            gain_c = _load_chunked_vec(nc, persist, gain_h[:], "gain")
            b0_c = _load_chunked_vec(nc, persist, b0_h[:], "b0")
            b1_c = _load_chunked_vec(nc, persist, b1_h[:], "b1")
            pa_c = _load_chunked_vec(nc, persist, pa_h[:], "pa")
            b2_c = _load_chunked_vec(nc, persist, b2_h[:], "b2")

            # derived per-channel epilogue constants:
            #   ab1b2 = a*b1 + b2  ;  b1b2 = b1 + b2
            ab1b2 = []
            b1b2 = []
            for c in range(NCH):
                ab = persist.tile([128, 1], FP32, tag=f"ab1b2{c}", name=f"ab1b2{c}")
                nc.vector.scalar_tensor_tensor(
                    out=ab, in0=b1_c[c], scalar=pa_c[c], in1=b2_c[c],
                    op0=mybir.AluOpType.mult, op1=mybir.AluOpType.add,
                )
                ab1b2.append(ab)
                bb = persist.tile([128, 1], FP32, tag=f"b1b2{c}", name=f"b1b2{c}")
                nc.vector.tensor_tensor(
                    out=bb, in0=b1_c[c], in1=b2_c[c], op=mybir.AluOpType.add
                )
                b1b2.append(bb)

            # ---- activation tiles: border-only memset --------------------
            # act_img[q][n] : [128, 2, ACT_IMG] fp8 -- two cin chunks per
            # DoubleRow pair; zeros on the padded ring + tail slack, interior
            # is fully overwritten by xsign.
            act_img = []
            for q in range(NPAIR):
                row = []
                for n in range(N_PER):
                    ap_t = persist.tile(
                        [128, 2, ACT_IMG], FP8, tag=f"act{q}_{n}", name=f"act{q}_{n}"
                    )
                    nc.gpsimd.memset(ap_t[:, :, 0:WP], 0.0)            # top row
                    nc.gpsimd.memset(ap_t[:, :, 29 * WP : ACT_IMG], 0.0)  # bottom+slack
                    mid = ap_t[:, :, WP : 29 * WP].rearrange(
                        "p h (r c) -> p h r c", c=WP
                    )
                    nc.gpsimd.memset(mid[:, :, :, 0:1], 0.0)           # left col
                    nc.gpsimd.memset(mid[:, :, :, 29:30], 0.0)         # right col
                    row.append(ap_t)
                act_img.append(row)

            xs_tiles = [
                persist.tile(
                    [128, N_PER, H, W], FP32, tag=f"xs{c}", name=f"xs{c}"
                )
                for c in range(NCH)
            ]
            xr = x_ap.rearrange("n c h w -> c n h w")

            # ---- DMA issue order: w0 first, x images next (halves for
            # queue parallelism), w1 before x3 so chunk-1 prep isn't late.
            w_flat = w_ap.rearrange("o i a b -> o (i a b)")
            w_tiles = {}

            def w_dma(m):
                wt = wraw.tile([128, KFAN], FP32, tag="wtile", name=f"wt{m}")
                for j in range(TAPS):
                    nc.sync.dma_start(
                        out=wt[:, j * 512 : (j + 1) * 512],
                        in_=w_flat[
                            m * 128 : (m + 1) * 128, j * 512 : (j + 1) * 512
                        ],
                    )
                w_tiles[m] = wt

            def x_dma(n):
                for c in range(NCH):
                    for hh in range(2):
                        r0 = hh * (H // 2)
                        nc.scalar.dma_start(
                            out=xs_tiles[c][:, n, r0 : r0 + H // 2],
                            in_=xr[c * 128 : (c + 1) * 128, n, r0 : r0 + H // 2],
                        )

            w_dma(0)
            x_dma(0)
            x_dma(1)
            x_dma(2)
            w_dma(1)
            x_dma(3)
            w_dma(2)
            w_dma(3)

            # ---- weight prep ---------------------------------------------
            # ws4[m]: [128 cout, 9 tap, 512 cin] bf16 sign(w - mean)
            # wsT per (m, t): [128 cin-off, 4 block, 128 cout] via DMA XBAR
            # lhsT: [128 cin, tap, pair, half, cout] fp8
            lhsT = persist.tile(
                [128, TAPS, NPAIR, 2, C], FP8, tag="lhsT", name="lhsT"
            )
            alphabar = {}
            negmeans = {}
            mvs = {}

            def bn_part(m, lo, hi):
                if m not in mvs:
                    st = stats.tile([128, TAPS, 6], FP32, tag="bnst", name=f"bnst{m}")
                    mvs[m] = (st, None)
                st, _ = mvs[m]
                wt3 = w_tiles[m].rearrange("p (a b) -> p a b", b=512)
                for sg in range(lo, hi):
                    nc.vector.bn_stats(out=st[:, sg, :], in_=wt3[:, sg, :])

            def bn_finish(m):
                st, _ = mvs[m]
                mv = stats.tile([128, 2], FP32, tag="bnagg", name=f"bnagg{m}")
                nc.vector.bn_aggr(out=mv, in_=st)
                negmean = stats.tile(
                    [128, 1], FP32, tag="negmean", name=f"negmean{m}"
                )
                nc.vector.tensor_scalar_mul(out=negmean, in0=mv[:, 0:1], scalar1=-1.0)
                mvs[m] = (st, mv)
                negmeans[m] = negmean

            def wsign_tap(m, ws4, t):
                """sign(w - mean) for tap t -> bf16, then XBAR-transpose and
                cast into lhsT (pair/half blocks of cout chunk m)."""
                # w free layout is (cin 512, tap 9): tap-t slice is strided
                wt3 = w_tiles[m].rearrange("p (c t) -> p c t", t=TAPS)
                nc.scalar.activation(
                    out=ws4[:, t, :],
                    in_=wt3[:, :, t],
                    func=mybir.ActivationFunctionType.Sign,
                    bias=negmeans[m],
                )
                tr = wst.tile([128, NCH, 128], BF16, tag="wst", name=f"wst{m}_{t}")
                nc.sync.dma_start_transpose(out=tr, in_=ws4[:, t, :])
                nc.gpsimd.tensor_copy(
                    out=lhsT[:, t, :, :, m * 128 : (m + 1) * 128],
                    in_=tr.rearrange("p (q h) c -> p q h c", q=NPAIR),
                )

            def wsign_all(m):
                ws4 = wsig.tile([128, TAPS, 512], BF16, tag="wsig", name=f"ws{m}")
                for t in range(TAPS):
                    wsign_tap(m, ws4, t)

            def absprep(m):
                """1/(std+eps), sum|w-mean| -> alphabar (incl. alpha, gain)."""
                wt = w_tiles[m]
                st, mv = mvs[m]
                negmean = negmeans[m]
                stdeps = stats.tile([128, 1], FP32, tag="stdeps", name=f"stdeps{m}")
                nc.scalar.activation(
                    out=stdeps, in_=mv[:, 1:2], func=mybir.ActivationFunctionType.Sqrt
                )
                nc.vector.tensor_scalar_add(out=stdeps, in0=stdeps, scalar1=EPS)
                inv = stats.tile([128, 1], FP32, tag="inv", name=f"inv{m}")
                nc.vector.reciprocal(out=inv, in_=stdeps)

                sumabs = stats.tile([128, NCH], FP32, tag="sumabs", name=f"sumabs{m}")
                for b in range(NCH):
                    nc.scalar.activation(
                        out=wt[:, b * 1152 : (b + 1) * 1152],
                        in_=wt[:, b * 1152 : (b + 1) * 1152],
                        func=mybir.ActivationFunctionType.Abs,
                        bias=negmean,
                        accum_out=sumabs[:, b : b + 1],
                    )
                sumabs1 = stats.tile([128, 1], FP32, tag="sumabs1", name=f"sumabs1{m}")
                nc.vector.tensor_reduce(
                    out=sumabs1, in_=sumabs, axis=mybir.AxisListType.X,
                    op=mybir.AluOpType.add,
                )
                ab = persist.tile(
                    [128, 1], FP32, tag=f"alphabar{m}", name=f"alphabar{m}"
                )
                nc.vector.tensor_tensor(
                    out=ab, in0=sumabs1, in1=inv, op=mybir.AluOpType.mult
                )
                nc.vector.tensor_tensor(
                    out=ab, in0=ab, in1=gain_c[m], op=mybir.AluOpType.mult
                )
                nc.vector.tensor_scalar_mul(
                    out=ab, in0=ab, scalar1=ALPHA * WS_SCALE / KFAN
                )
                alphabar[m] = ab

            def xsign(n, c):
                dst = act_img[c // 2][n][:, c % 2, : HP * WP].rearrange(
                    "p (h w) -> p h w", w=WP
                )[:, 1 : 1 + H, 1 : 1 + W]
                nc.scalar.activation(
                    out=dst,
                    in_=xs_tiles[c][:, n],
                    func=mybir.ActivationFunctionType.Sign,
                    bias=b0_c[c],
                    scale=BETA,
                )

            # ---- chunk-0 prep: ACT order is xsign(img0) -> wsign(0) -> abs(0)
            bn_part(0, 0, TAPS)
            bn_finish(0)
            for c in range(NCH):
                xsign(0, c)
            wsign_all(0)
            absprep(0)
            for c in range(NCH):
                xsign(1, c)

            # ---- conv + epilogue ----------------------------------------
            def conv_tile(m, n, h2, otile):
                y0 = h2 * ROWS_PER_TILE
                acc = psum_mm.tile([128, 512], FP32, tag="acc", name="acc")
                i = 0
                for q in range(NPAIR):
                    for t in range(TAPS):
                        dy, dx = t // 3, t % 3
                        base = (y0 + dy) * WP + dx
                        rhs = act_img[q][n][:, :, base : base + NFREE]
                        nc.tensor.matmul(
                            acc[:, :NFREE],
                            lhsT[:, t, q, :, m * 128 : (m + 1) * 128],
                            rhs,
                            start=(i == 0),
                            stop=(i == NPAIR * TAPS - 1),
                            perf_mode=mybir.MatmulPerfMode.DoubleRow,
                        )
                        i += 1
                accv = acc[:, :NFREE].rearrange("p (h w) -> p h w", w=WP)[
                    :, :, 0:W
                ]
                res = xs_tiles[m][:, n, y0 : y0 + ROWS_PER_TILE, :]
                # z = acc*alphabar + residual   (pre-b1 prelu input)
                z = epi.tile([128, ROWS_PER_TILE, W], FP32, tag="z", name="z")
                nc.vector.scalar_tensor_tensor(
                    out=z, in0=accv, scalar=alphabar[m], in1=res,
                    op0=mybir.AluOpType.mult, op1=mybir.AluOpType.add,
                )
                # u = a*z + (a*b1 + b2)     [GpSimd]
                u = epi.tile([128, ROWS_PER_TILE, W], FP32, tag="u", name="u")
                nc.gpsimd.tensor_scalar(
                    out=u, in0=z, scalar1=pa_c[m], scalar2=ab1b2[m],
                    op0=mybir.AluOpType.mult, op1=mybir.AluOpType.add,
                )
                # out = max(z + (b1+b2), u)  == prelu(z+b1) + b2  (a<=1)
                nc.vector.scalar_tensor_tensor(
                    out=otile[:, y0 : y0 + ROWS_PER_TILE, :],
                    in0=z, scalar=b1b2[m], in1=u,
                    op0=mybir.AluOpType.add, op1=mybir.AluOpType.max,
                )

            for m in range(NCH):
                for n in range(N_PER):
                    otile = opool.tile([128, H, W], FP32, tag="o", name=f"o{m}_{n}")
                    for h2 in range(NSPAT):
                        conv_tile(m, n, h2, otile)
                        nc.sync.dma_start(
                            out=out_ap[
                                n, m * 128 : (m + 1) * 128,
                                h2 * ROWS_PER_TILE : (h2 + 1) * ROWS_PER_TILE, :,
                            ],
                            in_=otile[
                                :, h2 * ROWS_PER_TILE : (h2 + 1) * ROWS_PER_TILE, :
                            ],
                        )
                    # next-chunk prep interleaved at image boundaries
                    if m == 0 and n == 0:
                        for c in range(NCH):
                            xsign(2, c)
                    if m == 0 and n == 1:
                        for c in range(NCH):
                            xsign(3, c)
                    if m + 1 < NCH:
                        if n == 0:
                            bn_part(m + 1, 0, 5)
                        elif n == 1:
                            bn_part(m + 1, 5, TAPS)
                            bn_finish(m + 1)
                            wsign_all(m + 1)
                        elif n == 2:
                            absprep(m + 1)

    nc.finalize()
    return nc


_NC_CACHE = None


def _get_program():
    global _NC_CACHE
    if _NC_CACHE is None:
        _NC_CACHE = build_program()
    return _NC_CACHE


def kernel(**inputs):
    from concourse.bass_utils import run_bass_kernel_spmd

    x = np.ascontiguousarray(np.asarray(inputs["x"], dtype=np.float32))
    shared = {
        name: np.ascontiguousarray(np.asarray(inputs[name], dtype=np.float32))
        for name in (
            "conv_weight", "gain", "move0_bias", "move1_bias", "prelu_a",
            "move2_bias",
        )
    }
    nc = _get_program()
    in_maps = [
        {"x": x[i * N_PER : (i + 1) * N_PER], **shared} for i in range(N_CORES)
    ]
    res = run_bass_kernel_spmd(nc, in_maps, core_ids=list(range(N_CORES)))
    return np.concatenate([r["out"] for r in res.results], axis=0)
